# revision 2
# baseline (speedup 1.0000x reference)
"""Distributed Trainium2 kernel for AdaptiveSimpleGCNConv.

Math: out = D^{-1/2} (A_set + I) D^{-1/2} @ x @ W.T + b
  A_set: dense 0/1 adjacency from edge_index (duplicates collapse), N=8192.

Strategy (8 NeuronCores, 1D row partition of nodes):
  - Host: dedup edges, compute degree/d=1/sqrt(deg), fold the column scale
    into x' = d*x, build per-core transposed adjacency shards (values 0/1/2,
    exact in bf16) in a DMA-friendly supertile layout.
  - Device k: yT = sum_c x'[c,:] ^T outer adjT[c, rows_k]  via PE accumulation
    (lhsT = x' chunk [128c x 128f], rhs = adjT supertile slice [128c x 512r]),
    then out = (yT.T @ W.T) * d_rows + b via a second matmul + fused epilogue.
  - No collectives: x' is replicated to every core by the host.
"""

import sys

sys.path.insert(0, "/opt/trn_rl_repo")

import numpy as np
import ml_dtypes

N = 8192
D = 128
NCORES = 8
RPC = N // NCORES  # 1024 rows per core
NCHUNK = N // 128  # 64 contraction chunks
NWIN = RPC // 512  # 2 row windows per core
SUPER = 8  # chunks per adjacency supertile DMA
BF16 = ml_dtypes.bfloat16

_CACHE = {}


def _build_nc():
    from concourse import bacc, bass, tile, mybir

    nc = bacc.Bacc("TRN2", target_bir_lowering=False, debug=False,
                   num_devices=NCORES)

    adjt_ext = nc.declare_dram_parameter(
        "adjT", [NWIN, 128, NCHUNK, 512], mybir.dt.bfloat16, isOutput=False)
    xc_ext = nc.declare_dram_parameter(
        "xc", [128, NCHUNK, D], mybir.dt.bfloat16, isOutput=False)
    wt_ext = nc.declare_dram_parameter(
        "wT", [D, D], mybir.dt.bfloat16, isOutput=False)
    bb_ext = nc.declare_dram_parameter(
        "bb", [128, D], mybir.dt.float32, isOutput=False)
    dr_ext = nc.declare_dram_parameter(
        "dr", [128, RPC // 128], mybir.dt.float32, isOutput=False)
    out_ext = nc.declare_dram_parameter(
        "out", [RPC, D], mybir.dt.float32, isOutput=True)

    with tile.TileContext(nc) as tc:
        with (
            tc.tile_pool(name="const", bufs=1) as constp,
            tc.tile_pool(name="adj", bufs=4) as adjp,
            tc.tile_pool(name="yt", bufs=2) as ytp,
            tc.tile_pool(name="ot", bufs=3) as otp,
            tc.tile_pool(name="ps_y", bufs=2, space=bass.MemorySpace.PSUM) as psy,
            tc.tile_pool(name="ps_o", bufs=2, space=bass.MemorySpace.PSUM) as pso,
        ):
            x_all = constp.tile([128, NCHUNK, D], mybir.dt.bfloat16)
            nc.sync.dma_start(out=x_all[:], in_=xc_ext[:])
            wt = constp.tile([D, D], mybir.dt.bfloat16)
            nc.sync.dma_start(out=wt[:], in_=wt_ext[:])
            bb = constp.tile([128, D], mybir.dt.float32)
            nc.sync.dma_start(out=bb[:], in_=bb_ext[:])
            dr = constp.tile([128, RPC // 128], mybir.dt.float32)
            nc.sync.dma_start(out=dr[:], in_=dr_ext[:])

            for w in range(NWIN):
                ps_y = psy.tile([128, 512], mybir.dt.float32)
                for s in range(NCHUNK // SUPER):
                    at = adjp.tile([128, SUPER, 512], mybir.dt.bfloat16,
                                   tag="adjtile")
                    nc.sync.dma_start(
                        out=at[:],
                        in_=adjt_ext[w, :, s * SUPER:(s + 1) * SUPER, :])
                    for j in range(SUPER):
                        cc = s * SUPER + j
                        nc.tensor.matmul(
                            ps_y[:],
                            lhsT=x_all[:, cc, :],
                            rhs=at[:, j, :],
                            start=(cc == 0),
                            stop=(cc == NCHUNK - 1),
                        )
                yt = ytp.tile([128, 512], mybir.dt.bfloat16)
                nc.vector.tensor_copy(yt[:], ps_y[:])
                for m in range(4):
                    g = w * 4 + m
                    ps_o = pso.tile([128, D], mybir.dt.float32)
                    nc.tensor.matmul(
                        ps_o[:],
                        lhsT=yt[:, m * 128:(m + 1) * 128],
                        rhs=wt[:],
                        start=True,
                        stop=True,
                    )
                    ot = otp.tile([128, D], mybir.dt.float32, tag="outtile")
                    nc.vector.scalar_tensor_tensor(
                        out=ot[:],
                        in0=ps_o[:],
                        scalar=dr[:, g:g + 1],
                        in1=bb[:],
                        op0=mybir.AluOpType.mult,
                        op1=mybir.AluOpType.add,
                    )
                    nc.sync.dma_start(out=out_ext[g * 128:(g + 1) * 128, :],
                                      in_=ot[:])
    nc.compile()
    return nc


def _host_prep(x, edge_index, W, b):
    r = np.asarray(edge_index[0]).astype(np.int64)
    c = np.asarray(edge_index[1]).astype(np.int64)
    uniq = np.unique(r * N + c)
    r_u = uniq // N
    c_u = uniq % N

    degree = np.bincount(r_u, minlength=N).astype(np.float64) + 1.0
    d = (1.0 / np.sqrt(degree)).astype(np.float32)

    xp = (np.asarray(x, dtype=np.float32) * d[:, None]).astype(BF16)
    xc = np.ascontiguousarray(
        xp.reshape(NCHUNK, 128, D).transpose(1, 0, 2))  # [128, chunk, feat]

    wt = np.ascontiguousarray(np.asarray(W, dtype=np.float32).T).astype(BF16)
    bb = np.ascontiguousarray(
        np.tile(np.asarray(b, dtype=np.float32)[None, :], (128, 1)))

    in_maps = []
    for k in range(NCORES):
        mask = (r_u // RPC) == k
        rr = r_u[mask] - k * RPC  # local row in [0, RPC)
        cs = c_u[mask]            # global col in [0, N)
        adjt = np.zeros((NWIN, 128, NCHUNK, 512), dtype=BF16)
        # adjt[w, p, cc, q] corresponds to adj[row = w*512+q (local), col = cc*128+p]
        adjt[rr >> 9, cs & 127, cs >> 7, rr & 511] = 1.0
        jj = np.arange(RPC)
        ii = k * RPC + jj  # global diag index -> column
        adjt[jj >> 9, ii & 127, ii >> 7, jj & 511] += np.ones(RPC, dtype=BF16)
        dr = np.ascontiguousarray(
            d[k * RPC:(k + 1) * RPC].reshape(RPC // 128, 128).T)
        in_maps.append({"adjT": adjt, "xc": xc, "wT": wt, "bb": bb, "dr": dr})
    return in_maps


def kernel(x, edge_index, W, b):
    from concourse.bass_utils import run_bass_kernel_spmd

    in_maps = _host_prep(x, edge_index, W, b)
    if "nc" not in _CACHE:
        _CACHE["nc"] = _build_nc()
    nc = _CACHE["nc"]
    res = run_bass_kernel_spmd(nc, in_maps, core_ids=list(range(NCORES)))
    out = np.concatenate(
        [np.asarray(res.results[k]["out"]) for k in range(NCORES)], axis=0)
    return np.ascontiguousarray(out.astype(np.float32))


if __name__ == "__main__":
    rng = np.random.default_rng(0)
    x = rng.standard_normal((N, D), dtype=np.float32)
    ei = rng.integers(0, N, size=(2, 262144)).astype(np.int64)
    W = rng.standard_normal((D, D), dtype=np.float32) / np.sqrt(D)
    b = rng.standard_normal(D, dtype=np.float32) * 0.01
    out = kernel(x=x, edge_index=ei, W=W, b=b)
    print(out.shape, out.dtype, float(np.abs(out).mean()))


# revision 4
# speedup vs baseline: 1.2023x; 1.2023x over previous
"""Distributed Trainium2 kernel for AdaptiveSimpleGCNConv.

Math: out = D^{-1/2} (A_set + I) D^{-1/2} @ x @ W.T + b
  A_set: dense 0/1 adjacency from edge_index (duplicates collapse), N=8192.

Strategy (8 NeuronCores, 1D row partition of nodes):
  - Host: dedup edges, compute degree/d=1/sqrt(deg), fold the column scale
    into x' = d*x, build per-core transposed adjacency shards (values 0/1/2,
    exact in fp8/bf16) in a DMA-friendly layout [col%128, col//128, row].
  - Device k: for each col-chunk c: psum_w += x'[c]^T-stationary matmul with
    adjT[c, rows] moving (chunk-major: one weight load serves both 512-row
    windows), then out = (y @ W.T) * d_rows + b via a second matmul + fused
    vector epilogue.
  - No collectives: x' is replicated to every core by the host.
"""

import sys

sys.path.insert(0, "/opt/trn_rl_repo")

import numpy as np
import ml_dtypes

N = 8192
D = 128
NCORES = 8
RPC = N // NCORES   # 1024 rows per core
NCHUNK = N // 128   # 64 contraction chunks
NWIN = RPC // 512   # 2 row windows per core
SUPER = 4           # chunks per adjacency supertile DMA
BF16 = ml_dtypes.bfloat16
FP8 = ml_dtypes.float8_e4m3fn

# adjacency storage dtype: "bf16" or "fp8"
ADJ_DTYPE = "fp8"

_CACHE = {}


def _build_nc(adj_dtype=ADJ_DTYPE):
    from concourse import bacc, bass, tile, mybir

    adt = mybir.dt.float8e4 if adj_dtype == "fp8" else mybir.dt.bfloat16

    nc = bacc.Bacc("TRN2", target_bir_lowering=False, debug=False,
                   num_devices=NCORES)

    adjt_ext = nc.declare_dram_parameter(
        "adjT", [128, NCHUNK, RPC], adt, isOutput=False)
    xc_ext = nc.declare_dram_parameter(
        "xc", [128, NCHUNK, D], mybir.dt.bfloat16, isOutput=False)
    wt_ext = nc.declare_dram_parameter(
        "wT", [D, D], mybir.dt.bfloat16, isOutput=False)
    bb_ext = nc.declare_dram_parameter(
        "bb", [128, D], mybir.dt.float32, isOutput=False)
    dr_ext = nc.declare_dram_parameter(
        "dr", [128, RPC // 128], mybir.dt.float32, isOutput=False)
    out_ext = nc.declare_dram_parameter(
        "out", [RPC, D], mybir.dt.float32, isOutput=True)

    with tile.TileContext(nc) as tc:
        with (
            tc.tile_pool(name="const", bufs=1) as constp,
            tc.tile_pool(name="adj", bufs=6) as adjp,
            tc.tile_pool(name="yt", bufs=2) as ytp,
            tc.tile_pool(name="ot", bufs=3) as otp,
            tc.tile_pool(name="ps_y", bufs=2, space=bass.MemorySpace.PSUM) as psy,
            tc.tile_pool(name="ps_o", bufs=2, space=bass.MemorySpace.PSUM) as pso,
        ):
            x_all = constp.tile([128, NCHUNK, D], mybir.dt.bfloat16)
            nc.sync.dma_start(out=x_all[:], in_=xc_ext[:])
            wt = constp.tile([D, D], mybir.dt.bfloat16)
            nc.sync.dma_start(out=wt[:], in_=wt_ext[:])
            bb = constp.tile([128, D], mybir.dt.float32)
            nc.sync.dma_start(out=bb[:], in_=bb_ext[:])
            dr = constp.tile([128, RPC // 128], mybir.dt.float32)
            nc.sync.dma_start(out=dr[:], in_=dr_ext[:])

            ps = [psy.tile([128, 512], mybir.dt.float32, tag=f"psw{w}",
                           name=f"ps_win{w}")
                  for w in range(NWIN)]
            for s in range(NCHUNK // SUPER):
                at = adjp.tile([128, SUPER, RPC], adt, tag="adjtile")
                nc.sync.dma_start(
                    out=at[:], in_=adjt_ext[:, s * SUPER:(s + 1) * SUPER, :])
                for j in range(SUPER):
                    cc = s * SUPER + j
                    for w in range(NWIN):
                        nc.tensor.matmul(
                            ps[w][:],
                            lhsT=x_all[:, cc, :],
                            rhs=at[:, j, w * 512:(w + 1) * 512],
                            start=(cc == 0),
                            stop=(cc == NCHUNK - 1),
                        )

            for w in range(NWIN):
                yt = ytp.tile([128, 512], mybir.dt.bfloat16)
                nc.vector.tensor_copy(yt[:], ps[w][:])
                for m in range(4):
                    g = w * 4 + m
                    ps_o = pso.tile([128, D], mybir.dt.float32)
                    nc.tensor.matmul(
                        ps_o[:],
                        lhsT=yt[:, m * 128:(m + 1) * 128],
                        rhs=wt[:],
                        start=True,
                        stop=True,
                    )
                    ot = otp.tile([128, D], mybir.dt.float32, tag="outtile")
                    nc.vector.scalar_tensor_tensor(
                        out=ot[:],
                        in0=ps_o[:],
                        scalar=dr[:, g:g + 1],
                        in1=bb[:],
                        op0=mybir.AluOpType.mult,
                        op1=mybir.AluOpType.add,
                    )
                    nc.sync.dma_start(out=out_ext[g * 128:(g + 1) * 128, :],
                                      in_=ot[:])
    nc.compile()
    return nc


def _host_prep(x, edge_index, W, b, adj_dtype=ADJ_DTYPE):
    npdt = FP8 if adj_dtype == "fp8" else BF16
    r = np.asarray(edge_index[0]).astype(np.int64)
    c = np.asarray(edge_index[1]).astype(np.int64)
    uniq = np.unique(r * N + c)
    r_u = uniq // N
    c_u = uniq % N

    degree = np.bincount(r_u, minlength=N).astype(np.float64) + 1.0
    d = (1.0 / np.sqrt(degree)).astype(np.float32)

    xp = (np.asarray(x, dtype=np.float32) * d[:, None]).astype(BF16)
    xc = np.ascontiguousarray(
        xp.reshape(NCHUNK, 128, D).transpose(1, 0, 2))  # [128, chunk, feat]

    wt = np.ascontiguousarray(np.asarray(W, dtype=np.float32).T).astype(BF16)
    bb = np.ascontiguousarray(
        np.tile(np.asarray(b, dtype=np.float32)[None, :], (128, 1)))

    in_maps = []
    for k in range(NCORES):
        mask = (r_u // RPC) == k
        rr = r_u[mask] - k * RPC  # local row in [0, RPC)
        cs = c_u[mask]            # global col in [0, N)
        adjt = np.zeros((128, NCHUNK, RPC), dtype=npdt)
        # adjt[p, cc, q] corresponds to adj[row = q (local), col = cc*128+p]
        adjt[cs & 127, cs >> 7, rr] = 1.0
        jj = np.arange(RPC)
        ii = k * RPC + jj  # global diag index -> column
        adjt[ii & 127, ii >> 7, jj] += np.ones(RPC, dtype=npdt)
        dr = np.ascontiguousarray(
            d[k * RPC:(k + 1) * RPC].reshape(RPC // 128, 128).T)
        in_maps.append({"adjT": adjt, "xc": xc, "wT": wt, "bb": bb, "dr": dr})
    return in_maps


def kernel(x, edge_index, W, b):
    from concourse.bass_utils import run_bass_kernel_spmd

    in_maps = _host_prep(x, edge_index, W, b)
    if "nc" not in _CACHE:
        _CACHE["nc"] = _build_nc()
    nc = _CACHE["nc"]
    res = run_bass_kernel_spmd(nc, in_maps, core_ids=list(range(NCORES)))
    out = np.concatenate(
        [np.asarray(res.results[k]["out"]) for k in range(NCORES)], axis=0)
    return np.ascontiguousarray(out.astype(np.float32))


if __name__ == "__main__":
    rng = np.random.default_rng(0)
    x = rng.standard_normal((N, D), dtype=np.float32)
    ei = rng.integers(0, N, size=(2, 262144)).astype(np.int64)
    W = rng.standard_normal((D, D), dtype=np.float32) / np.sqrt(D)
    b = rng.standard_normal(D, dtype=np.float32) * 0.01
    out = kernel(x=x, edge_index=ei, W=W, b=b)
    print(out.shape, out.dtype, float(np.abs(out).mean()))


# revision 5
# speedup vs baseline: 1.3323x; 1.1082x over previous
"""Distributed Trainium2 kernel for AdaptiveSimpleGCNConv.

Math: out = D^{-1/2} (A_set + I) D^{-1/2} @ x @ W.T + b
  A_set: dense 0/1 adjacency from edge_index (duplicates collapse), N=8192.

Strategy (8 NeuronCores, 1D row partition of nodes):
  - Host: dedup edges, compute degree/d=1/sqrt(deg), fold the column scale
    into x' = d*x, build per-core transposed adjacency shards (values 0/1/2,
    exact in fp8/bf16) in a DMA-friendly layout [col%128, col//128, row].
  - Device k: for each col-chunk c: psum_w += x'[c]^T-stationary matmul with
    adjT[c, rows] moving (chunk-major: one weight load serves both 512-row
    windows), then out = (y @ W.T) * d_rows + b via a second matmul + fused
    vector epilogue.
  - No collectives: x' is replicated to every core by the host.
"""

import sys

sys.path.insert(0, "/opt/trn_rl_repo")

import numpy as np
import ml_dtypes

N = 8192
D = 128
NCORES = 8
RPC = N // NCORES   # 1024 rows per core
NCHUNK = N // 128   # 64 contraction chunks
NWIN = RPC // 512   # 2 row windows per core
SUPER = 4           # chunks per adjacency supertile DMA
BF16 = ml_dtypes.bfloat16
FP8 = ml_dtypes.float8_e4m3fn

# adjacency storage dtype: "bf16" or "fp8"
ADJ_DTYPE = "fp8"

_CACHE = {}


N_WARMUP = 16   # dummy matmuls to lift the PE out of the cold p-state
XPIECE = 8      # chunks per x-load piece


def _patch_ldw_opt():
    """Flip walrus's --enable-ldw-opt to true (dedups/overlaps LDWEIGHTS)."""
    import os
    if os.environ.get("LDWOPT", "0") != "1":
        return
    from concourse import bass_utils as bu
    orig = bu.run_command

    def patched(cmd, *a, **kw):
        cmd = [c.replace("--enable-ldw-opt=false", "--enable-ldw-opt=true")
               if isinstance(c, str) else c for c in cmd]
        return orig(cmd, *a, **kw)

    bu.run_command = patched


def _build_nc(adj_dtype=ADJ_DTYPE):
    from concourse import bacc, bass, tile, mybir

    _patch_ldw_opt()
    adt = mybir.dt.float8e4 if adj_dtype == "fp8" else mybir.dt.bfloat16

    nc = bacc.Bacc("TRN2", target_bir_lowering=False, debug=False,
                   num_devices=NCORES)

    adjt_ext = nc.declare_dram_parameter(
        "adjT", [128, NCHUNK, RPC], adt, isOutput=False)
    xc_ext = nc.declare_dram_parameter(
        "xc", [128, NCHUNK, D], mybir.dt.bfloat16, isOutput=False)
    wt_ext = nc.declare_dram_parameter(
        "wT", [D, D], mybir.dt.bfloat16, isOutput=False)
    bb_ext = nc.declare_dram_parameter(
        "bb", [128, D], mybir.dt.float32, isOutput=False)
    dr_ext = nc.declare_dram_parameter(
        "dr", [128, RPC // 128], mybir.dt.float32, isOutput=False)
    out_ext = nc.declare_dram_parameter(
        "out", [RPC, D], mybir.dt.float32, isOutput=True)

    NSUPER = NCHUNK // SUPER

    with tile.TileContext(nc) as tc:
        with (
            tc.tile_pool(name="const", bufs=1) as constp,
            tc.tile_pool(name="adj", bufs=6) as adjp,
            tc.tile_pool(name="yt", bufs=2) as ytp,
            tc.tile_pool(name="ot", bufs=3) as otp,
            tc.tile_pool(name="ps_y", bufs=2, space=bass.MemorySpace.PSUM) as psy,
            tc.tile_pool(name="ps_o", bufs=2, space=bass.MemorySpace.PSUM) as pso,
            tc.tile_pool(name="ps_j", bufs=1, space=bass.MemorySpace.PSUM) as psj,
        ):
            # PE warmup: junk matmuls with no data deps keep the PE busy
            # during the initial DMA so real matmuls start at full clock.
            junk_w = constp.tile([128, D], mybir.dt.bfloat16)
            junk_m = constp.tile([128, 512], mybir.dt.bfloat16)
            nc.vector.memset(junk_w[:], 0.0)
            nc.vector.memset(junk_m[:], 0.0)
            ps_junk = psj.tile([128, 512], mybir.dt.float32)
            for _ in range(N_WARMUP):
                nc.tensor.matmul(ps_junk[:], lhsT=junk_w[:], rhs=junk_m[:],
                                 start=True, stop=True)

            xp = [constp.tile([128, XPIECE, D], mybir.dt.bfloat16,
                              name=f"xpiece{i}", tag=f"xpiece{i}")
                  for i in range(NCHUNK // XPIECE)]
            for i, t in enumerate(xp):
                nc.sync.dma_start(
                    out=t[:], in_=xc_ext[:, i * XPIECE:(i + 1) * XPIECE, :])
            wt = constp.tile([D, D], mybir.dt.bfloat16)
            nc.sync.dma_start(out=wt[:], in_=wt_ext[:])
            bb = constp.tile([128, D], mybir.dt.float32)
            nc.sync.dma_start(out=bb[:], in_=bb_ext[:])
            dr = constp.tile([128, RPC // 128], mybir.dt.float32)
            nc.sync.dma_start(out=dr[:], in_=dr_ext[:])

            def xs(cc):
                return xp[cc // XPIECE][:, cc % XPIECE, :]

            ps = [psy.tile([128, 512], mybir.dt.float32, tag=f"psw{w}",
                           name=f"ps_win{w}")
                  for w in range(NWIN)]

            def mm(cc, w, at, j):
                nc.tensor.matmul(
                    ps[w][:],
                    lhsT=xs(cc),
                    rhs=at[:, j, w * 512:(w + 1) * 512],
                    start=(cc == 0),
                    stop=(cc == NCHUNK - 1),
                )

            for s in range(NSUPER):
                at = adjp.tile([128, SUPER, RPC], adt, tag="adjtile")
                nc.sync.dma_start(
                    out=at[:], in_=adjt_ext[:, s * SUPER:(s + 1) * SUPER, :])
                if s < NSUPER - 1:
                    for j in range(SUPER):
                        for w in range(NWIN):
                            mm(s * SUPER + j, w, at, j)
                else:
                    # last supertile window-major: window 0 finishes early so
                    # its epilogue overlaps window 1's tail matmuls
                    for w in range(NWIN):
                        for j in range(SUPER):
                            mm(s * SUPER + j, w, at, j)

            for w in range(NWIN):
                yt = ytp.tile([128, 512], mybir.dt.bfloat16)
                for m in range(4):
                    g = w * 4 + m
                    sl = slice(m * 128, (m + 1) * 128)
                    nc.vector.tensor_copy(yt[:, sl], ps[w][:, sl])
                    ps_o = pso.tile([128, D], mybir.dt.float32)
                    nc.tensor.matmul(
                        ps_o[:],
                        lhsT=yt[:, sl],
                        rhs=wt[:],
                        start=True,
                        stop=True,
                    )
                    ot = otp.tile([128, D], mybir.dt.float32, tag="outtile")
                    nc.vector.scalar_tensor_tensor(
                        out=ot[:],
                        in0=ps_o[:],
                        scalar=dr[:, g:g + 1],
                        in1=bb[:],
                        op0=mybir.AluOpType.mult,
                        op1=mybir.AluOpType.add,
                    )
                    nc.sync.dma_start(out=out_ext[g * 128:(g + 1) * 128, :],
                                      in_=ot[:])
    nc.compile()
    return nc


def _host_prep(x, edge_index, W, b, adj_dtype=ADJ_DTYPE):
    npdt = FP8 if adj_dtype == "fp8" else BF16
    r = np.asarray(edge_index[0]).astype(np.int64)
    c = np.asarray(edge_index[1]).astype(np.int64)
    uniq = np.unique(r * N + c)
    r_u = uniq // N
    c_u = uniq % N

    degree = np.bincount(r_u, minlength=N).astype(np.float64) + 1.0
    d = (1.0 / np.sqrt(degree)).astype(np.float32)

    xp = (np.asarray(x, dtype=np.float32) * d[:, None]).astype(BF16)
    xc = np.ascontiguousarray(
        xp.reshape(NCHUNK, 128, D).transpose(1, 0, 2))  # [128, chunk, feat]

    wt = np.ascontiguousarray(np.asarray(W, dtype=np.float32).T).astype(BF16)
    bb = np.ascontiguousarray(
        np.tile(np.asarray(b, dtype=np.float32)[None, :], (128, 1)))

    in_maps = []
    for k in range(NCORES):
        mask = (r_u // RPC) == k
        rr = r_u[mask] - k * RPC  # local row in [0, RPC)
        cs = c_u[mask]            # global col in [0, N)
        adjt = np.zeros((128, NCHUNK, RPC), dtype=npdt)
        # adjt[p, cc, q] corresponds to adj[row = q (local), col = cc*128+p]
        adjt[cs & 127, cs >> 7, rr] = 1.0
        jj = np.arange(RPC)
        ii = k * RPC + jj  # global diag index -> column
        adjt[ii & 127, ii >> 7, jj] += np.ones(RPC, dtype=npdt)
        dr = np.ascontiguousarray(
            d[k * RPC:(k + 1) * RPC].reshape(RPC // 128, 128).T)
        in_maps.append({"adjT": adjt, "xc": xc, "wT": wt, "bb": bb, "dr": dr})
    return in_maps


def kernel(x, edge_index, W, b):
    from concourse.bass_utils import run_bass_kernel_spmd

    in_maps = _host_prep(x, edge_index, W, b)
    if "nc" not in _CACHE:
        _CACHE["nc"] = _build_nc()
    nc = _CACHE["nc"]
    res = run_bass_kernel_spmd(nc, in_maps, core_ids=list(range(NCORES)))
    out = np.concatenate(
        [np.asarray(res.results[k]["out"]) for k in range(NCORES)], axis=0)
    return np.ascontiguousarray(out.astype(np.float32))


if __name__ == "__main__":
    rng = np.random.default_rng(0)
    x = rng.standard_normal((N, D), dtype=np.float32)
    ei = rng.integers(0, N, size=(2, 262144)).astype(np.int64)
    W = rng.standard_normal((D, D), dtype=np.float32) / np.sqrt(D)
    b = rng.standard_normal(D, dtype=np.float32) * 0.01
    out = kernel(x=x, edge_index=ei, W=W, b=b)
    print(out.shape, out.dtype, float(np.abs(out).mean()))
